# revision 15
# baseline (speedup 1.0000x reference)
"""Trainium2 Bass kernel for EvalBspPrime: derivative of degree-16 B-spline
(Bernstein) basis on [0,1].

out[n, m] = 16 * (b_{m-1}(y) - b_m(y)),  b_k(y) = C(15,k) y^k (1-y)^{15-k}

Factorized form used here (all constants exact in fp32, no cancellation):
  for 1 <= m <= 15:
      out_m = -16 * S_m * h^{a_m} * x^{b_m} * (u - m/16)
  with S_m = C(15,m-1)+C(15,m), h = u(1-u), and (h^a, x^b) a pair of stored
  even powers of u / v=(1-u) / p=-h.  out_0 = -16 v^15, out_16 = 16 u^15.

Sharding: pure elementwise over N -> N split across 8 cores (data parallel),
each core computes its block independently; no collectives.
"""

import os
import sys

import numpy as np

for _p in ("/opt/trn_rl_repo",):
    if os.path.isdir(_p) and _p not in sys.path:
        sys.path.insert(0, _p)

import concourse.bass as bass
import concourse.bacc as bacc
import concourse.mybir as mybir
from concourse import tile
from concourse.bass_utils import run_bass_kernel_spmd

P = 128          # SBUF partitions
COLS = 3907      # free-dim columns per core
CORE_PTS = P * COLS          # 500096 points per core
NCORES = 8
PAD_N = CORE_PTS * NCORES    # 4000768
K = 17           # ORDER + 1 outputs per point
FMAX = 512       # free-dim tile size

_BINOM = [1.0, 15.0, 105.0, 455.0, 1365.0, 3003.0, 5005.0, 6435.0,
          6435.0, 5005.0, 3003.0, 1365.0, 455.0, 105.0, 15.0, 1.0]

# m -> (a = exponent of h, first power operand, second power operand)
_BUILDS = {
    1: (0, "v12", "v2"), 2: (1, "p", "v12"), 3: (2, "p2", "v10"),
    4: (3, "p3", "v8"), 5: (4, "p4", "v6"), 6: (5, "p5", "v4"),
    7: (6, "p6", "v2"), 8: (7, "p3", "p4"), 9: (6, "p6", "u2"),
    10: (5, "p5", "u4"), 11: (4, "p4", "u6"), 12: (3, "p3", "u8"),
    13: (2, "p2", "u10"), 14: (1, "p", "u12"), 15: (0, "u12", "u2"),
}

# Builds computed as plain tensor_tensor on GPSIMD (safe stock Q7 kernel);
# their scale constant moves into the final scalar_tensor_tensor on DVE and
# the (u + gamma) factor is materialized on the scalar engine.
# Empty: GPSIMD shares an SBUF port with the DVE, and concurrent GPSIMD
# traffic measured DVE 2-tensor ops at ~2.4 cyc/elem instead of ~1.
_GPSIMD_BUILD_MS = set()


def _chunks():
    out = []
    c0 = 0
    while c0 < COLS:
        f = min(FMAX, COLS - c0)
        out.append((c0, f))
        c0 += f
    return out


def _build_nc():
    nc = bacc.Bacc("TRN2", target_bir_lowering=False, debug=False,
                   num_devices=NCORES)
    f32 = mybir.dt.float32
    AF = mybir.ActivationFunctionType
    OP = mybir.AluOpType
    X = nc.declare_dram_parameter("x", [P, COLS], f32, isOutput=False)
    OUT = nc.declare_dram_parameter("out", [P, COLS * K], f32, isOutput=True)

    # Register extra activation-bias constants the same way Bass.__init__
    # registers 0.0/1.0 — memset + barrier before the Tile region so the
    # activations that use them carry no scheduled dependency.
    for _v in (-0.5, -0.25):
        _t = nc.alloc_sbuf_tensor(f"const-float32-{_v}", [128, 1], f32)
        nc.gpsimd.memset(_t.ap(), _v)
        nc.const_aps.aps[(f32, _v)] = _t.ap()
    nc.all_engine_barrier()

    with tile.TileContext(nc) as tc:
        with (
            tc.tile_pool(name="io", bufs=2) as io_pool,
            tc.tile_pool(name="pw", bufs=2) as pw,
            tc.tile_pool(name="wb", bufs=6) as wb,
            tc.tile_pool(name="db", bufs=6) as db,
            tc.tile_pool(name="ob", bufs=2) as ob,
        ):
            for (c0, F) in _chunks():
                t = {}
                u = io_pool.tile([P, F], f32, tag="u")
                nc.sync.dma_start(u[:], X[:, c0:c0 + F])
                t["u"] = u

                def act_sq(dst, src, scale=1.0, bias=0.0):
                    tl = pw.tile([P, F], f32, tag=dst)
                    nc.scalar.activation(tl[:], src[:], AF.Square,
                                         bias=float(bias), scale=float(scale))
                    t[dst] = tl

                def g_mul(dst, a, b):
                    tl = pw.tile([P, F], f32, tag=dst)
                    nc.gpsimd.tensor_tensor(tl[:], t[a][:], t[b][:], OP.mult)
                    t[dst] = tl

                def v_mul(dst, a, b):
                    tl = pw.tile([P, F], f32, tag=dst)
                    nc.vector.tensor_tensor(tl[:], t[a][:], t[b][:], OP.mult)
                    t[dst] = tl

                def d_factor(m):
                    # D_m = sigma_m * (u - m/16) on ACT (affine Copy)
                    if m == 0:
                        sc, bi = 16.0, -16.0          # -16*(1-u)
                    elif m == 16:
                        sc, bi = 16.0, 0.0            # 16*u
                    else:
                        a = _BUILDS[m][0]
                        S = _BINOM[m - 1] + _BINOM[m]
                        sc = -16.0 * S * ((-1.0) ** a)
                        bi = -sc * (m / 16.0)
                    dm = db.tile([P, F], f32, tag="d")
                    nc.scalar.activation(dm[:], u[:], AF.Copy,
                                         bias=float(bi), scale=float(sc))
                    return dm

                # --- scalar-engine power ladder (squares of affine inputs) ---
                act_sq("q", u, 1.0, -0.5)            # (u - 1/2)^2
                tl = pw.tile([P, F], f32, tag="p")   # p = u^2 - u = -u(1-u)
                nc.scalar.activation(tl[:], t["q"][:], AF.Copy, bias=-0.25, scale=1.0)
                t["p"] = tl
                act_sq("p2", t["q"], 1.0, -0.25)     # (q - 1/4)^2 = p^2
                act_sq("u2", u)
                act_sq("v2", u, -1.0, 1.0)           # (1-u)^2
                act_sq("u4", t["u2"])
                act_sq("v4", t["v2"])
                act_sq("u8", t["u4"])
                act_sq("v8", t["v4"])

                # --- 2-tensor power products ---
                v_mul("p3", "p", "p2")
                v_mul("p5", "p2", "p3")
                v_mul("u6", "u2", "u4")
                v_mul("u10", "u2", "u8")
                v_mul("v6", "v2", "v4")
                v_mul("v10", "v2", "v8")
                act_sq("p4", t["p2"])
                act_sq("p6", t["p3"])
                act_sq("u12", t["u6"])
                act_sq("v12", t["v6"])

                out_t = ob.tile([P, F * K], f32, tag="out")
                out_r = out_t[:].rearrange("p (f k) -> p k f", k=K)

                # builds: W_m = P1*P2 (plain TT); finals: out_m = D_m * W_m
                pair = dict(_BUILDS)
                pair[0] = (0, "v12", "v2")    # v^14
                pair[16] = (0, "u12", "u2")   # u^14
                for m in range(17):
                    _, p1, p2n = pair[m]
                    w = wb.tile([P, F], f32, tag="w")
                    dm = d_factor(m)
                    nc.vector.tensor_tensor(w[:], t[p1][:], t[p2n][:], OP.mult)
                    nc.vector.tensor_tensor(
                        out_r[:, m, :], dm[:], w[:], OP.mult)

                nc.sync.dma_start(OUT[:, c0 * K:(c0 + F) * K], out_t[:])
    nc.finalize()
    return nc


_CACHE = {}


def _run(x, trace=False, trace_kwargs=None):
    x = np.ascontiguousarray(np.asarray(x, dtype=np.float32))
    n = x.shape[0]
    xf = x.reshape(-1)
    pad = PAD_N - n
    if pad:
        xf = np.concatenate([xf, np.full(pad, 0.5, np.float32)])
    shards = xf.reshape(NCORES, P, COLS)
    if "nc" not in _CACHE:
        _CACHE["nc"] = _build_nc()
    nc = _CACHE["nc"]
    in_maps = [{"x": np.ascontiguousarray(shards[i])} for i in range(NCORES)]
    kw = {}
    if trace:
        kw["trace"] = True
        if trace_kwargs:
            kw.update(trace_kwargs)
    res = run_bass_kernel_spmd(nc, in_maps, list(range(NCORES)), **kw)
    outs = res.results
    full = np.concatenate(
        [np.asarray(outs[i]["out"]).reshape(CORE_PTS, K) for i in range(NCORES)],
        axis=0)
    return full[:n], res


def kernel(x):
    out, _ = _run(x)
    return out


# revision 16
# speedup vs baseline: 1.2980x; 1.2980x over previous
"""Trainium2 Bass kernel for EvalBspPrime: derivative of degree-16 B-spline
(Bernstein) basis on [0,1].

out[n, m] = 16 * (b_{m-1}(y) - b_m(y)),  b_k(y) = C(15,k) y^k (1-y)^{15-k}

Factorized form (stable, all-multiplicative, no cancellation):
  out_m = sigma_m * (u - m/16) * E_{m-1},  sigma_m = -16*(C(15,m-1)+C(15,m))
  E_j   = u^j v^{14-j}  (degree-14 mixed monomials, v = 1-u)
with edges out_0 = -16 v^15 = (-v/16) * (256 E_0),
           out_16 = 16 u^15 = (u/16) * (256 E_14).

E ladder: G (deg 3) -> F = G*{v^4,u^4} (deg 7) -> E_even = Square(F) on the
scalar engine (|sigma| folded in via the Square's input scale), E_odd =
F_i*F_{i+1} on the vector engine. All output writes are contiguous
(k-major); the host interleaves to [N, 17] at the end.

Sharding: elementwise over N -> N/8 per core (data parallel), no
communication. GPSIMD is left idle on purpose: it shares an SBUF port with
the DVE and concurrent use measured DVE 2-tensor ops at ~2x cost.
"""

import math
import os
import sys

import numpy as np

for _p in ("/opt/trn_rl_repo",):
    if os.path.isdir(_p) and _p not in sys.path:
        sys.path.insert(0, _p)

import concourse.bacc as bacc
import concourse.bass as bass
import concourse.mybir as mybir
from concourse import tile
from concourse.bass_utils import run_bass_kernel_spmd

P = 128          # SBUF partitions
COLS = 3907      # free-dim columns per core
CORE_PTS = P * COLS          # 500096 points per core
NCORES = 8
PAD_N = CORE_PTS * NCORES    # 4000768
K = 17           # ORDER + 1 outputs per point
FMAX = 512       # free-dim tile size

_BINOM = [1.0, 15.0, 105.0, 455.0, 1365.0, 3003.0, 5005.0, 6435.0,
          6435.0, 5005.0, 3003.0, 1365.0, 455.0, 105.0, 15.0, 1.0]
# |sigma_m| = 16*(C(15,m-1)+C(15,m)) for m=1..15
_ASIG = [16.0 * (_BINOM[m - 1] + _BINOM[m]) for m in range(1, 16)]


def _chunks():
    out = []
    c0 = 0
    while c0 < COLS:
        f = min(FMAX, COLS - c0)
        out.append((c0, f))
        c0 += f
    return out


def _build_nc():
    nc = bacc.Bacc("TRN2", target_bir_lowering=False, debug=False,
                   num_devices=NCORES)
    f32 = mybir.dt.float32
    AF = mybir.ActivationFunctionType
    OP = mybir.AluOpType
    X = nc.declare_dram_parameter("x", [P, COLS], f32, isOutput=False)
    # k-major output: [P, 17 * COLS]; host interleaves to [N, 17]
    OUT = nc.declare_dram_parameter("out", [P, K * COLS], f32, isOutput=True)
    OUT_K = OUT.ap().rearrange("p (k c) -> p k c", k=K)

    with tile.TileContext(nc) as tc:
        with (
            tc.tile_pool(name="io", bufs=2) as io_pool,
            tc.tile_pool(name="pw", bufs=2) as pw,
            tc.tile_pool(name="wb", bufs=6) as wb,
            tc.tile_pool(name="ob", bufs=2) as ob,
        ):
            for (c0, F) in _chunks():
                t = {}
                u = io_pool.tile([P, F], f32, tag="u")
                nc.sync.dma_start(u[:], X[:, c0:c0 + F])
                t["u"] = u

                def act(dst, src, func, scale=1.0, bias=0.0):
                    tl = pw.tile([P, F], f32, tag=dst)
                    nc.scalar.activation(tl[:], src[:], func,
                                         bias=float(bias), scale=float(scale))
                    t[dst] = tl

                def v_mul(dst, a, b):
                    tl = pw.tile([P, F], f32, tag=dst)
                    nc.vector.tensor_tensor(tl[:], t[a][:], t[b][:], OP.mult)
                    t[dst] = tl

                # --- scalar-engine unary ladder ---
                act("v", u, AF.Copy, -1.0, 1.0)        # v = 1-u
                act("nu", u, AF.Copy, -1.0, 0.0)       # -u
                act("vo16", u, AF.Copy, 1.0 / 16.0, -1.0 / 16.0)  # (u-1)/16 = -v/16
                act("uo16", u, AF.Copy, 1.0 / 16.0, 0.0)          # u/16
                act("u2", u, AF.Square)
                act("v2", u, AF.Square, -1.0, 1.0)     # (1-u)^2
                act("u4", t["u2"], AF.Square)
                act("v4", t["v2"], AF.Square)

                # --- degree-3 monomials (DVE) ---
                v_mul("g0", "v", "v2")    # v^3
                v_mul("g1", "u", "v2")    # u v^2
                v_mul("g2", "v", "u2")    # u^2 v
                v_mul("g3", "u", "u2")    # u^3

                # --- degree-7 monomials F_i = u^i v^(7-i) (DVE) ---
                for i in range(4):
                    v_mul(f"f{i}", f"g{i}", "v4")
                for i in range(4):
                    v_mul(f"f{i + 4}", f"g{i}", "u4")

                # --- scaled degree-14 monomials Etil_j = |sigma_{j+1}| E_j ---
                # even j: Square(sqrt|sigma| * F_{j/2}) on ACT
                for j in range(0, 15, 2):
                    s = math.sqrt(_ASIG[j])
                    act(f"e{j}", t[f"f{j // 2}"], AF.Square, s, 0.0)

                out_t = ob.tile([P, F * K], f32, tag="out")

                def out_slice(m):
                    return out_t[:, m * F:(m + 1) * F]

                # odd j: (F_a * |sigma|) * F_{a+1} on DVE (STT), then finals.
                # Emit odd-E and finals interleaved so tiles free quickly.
                for j in range(1, 14, 2):
                    a = (j - 1) // 2
                    tl = wb.tile([P, F], f32, tag="w")
                    nc.vector.scalar_tensor_tensor(
                        tl[:], t[f"f{a}"][:], _ASIG[j], t[f"f{a + 1}"][:],
                        OP.mult, OP.mult)
                    t[f"e{j}"] = tl

                # finals: out_m = (-u + m/16) * Etil_{m-1}, m = 1..15
                for m in range(1, 16):
                    nc.vector.scalar_tensor_tensor(
                        out_slice(m), t["nu"][:], m / 16.0, t[f"e{m - 1}"][:],
                        OP.add, OP.mult)
                # out_0 = (-v/16) * Etil_0;  out_16 = (u/16) * Etil_14
                nc.vector.tensor_tensor(
                    out_slice(0), t["vo16"][:], t["e0"][:], OP.mult)
                nc.vector.tensor_tensor(
                    out_slice(16), t["uo16"][:], t["e14"][:], OP.mult)

                nc.sync.dma_start(OUT_K[:, :, c0:c0 + F],
                                  out_t[:].rearrange("p (k c) -> p k c", k=K))
    nc.finalize()
    return nc


_CACHE = {}


def _run(x, trace=False, trace_kwargs=None):
    x = np.ascontiguousarray(np.asarray(x, dtype=np.float32))
    n = x.shape[0]
    xf = x.reshape(-1)
    pad = PAD_N - n
    if pad:
        xf = np.concatenate([xf, np.full(pad, 0.5, np.float32)])
    shards = xf.reshape(NCORES, P, COLS)
    if "nc" not in _CACHE:
        _CACHE["nc"] = _build_nc()
    nc = _CACHE["nc"]
    in_maps = [{"x": np.ascontiguousarray(shards[i])} for i in range(NCORES)]
    kw = {}
    if trace:
        kw["trace"] = True
        if trace_kwargs:
            kw.update(trace_kwargs)
    res = run_bass_kernel_spmd(nc, in_maps, list(range(NCORES)), **kw)
    outs = res.results
    full = np.empty((PAD_N, K), dtype=np.float32)
    for i in range(NCORES):
        o = np.asarray(outs[i]["out"]).reshape(P, K, COLS)
        # [P, K, COLS] -> [P, COLS, K] -> [CORE_PTS, K]
        full[i * CORE_PTS:(i + 1) * CORE_PTS] = (
            o.transpose(0, 2, 1).reshape(CORE_PTS, K))
    return full[:n], res


def kernel(x):
    out, _ = _run(x)
    return out
